# revision 9
# baseline (speedup 1.0000x reference)
"""MDLSTM cell (2-direction) Bass/Tile kernel for Trainium2, 8-core SPMD.

Math (per direction d, with shared input projections):
    i = sigmoid(w_ii @ x + w_hi @ h_d + b_i)
    f = sigmoid(w_if @ x + w_hf @ h_d + b_f)
    g = tanh   (w_ig @ x + w_hg @ h_d + b_g)
    o = sigmoid(w_io @ x + w_ho @ h_d + b_o)
    c_d = f * c_prev_d + i * g
    h_d = o * tanh(c_d)
ct = ws0 * c_0 + ws1 * c_1 ;  ht = ws0 * h_0 + ws1 * h_1

Sharding: all activations/states split along N (=8192) across 8 cores;
weights replicated. No cross-core communication.

Per-core kernel: per output row tile (M=128) the 4 shared input
projections are computed once into PSUM (start=True groups) and copied to
SBUF; each of the 8 gate/direction accumulations then starts by injecting
that x-projection into its PSUM bank via a VectorE copy and accumulates
the 8 hidden-projection K-tiles on top (start=False matmuls — PE-write
accumulate onto engine-written PSUM, valid because every bank's first
group in program order is a start=True group that defines has_written).
ScalarE applies sigmoid/tanh + per-partition bias straight out of PSUM;
VectorE does the elementwise cell update and direction combine. Matmul
operands use float32r (fp32 storage, single-pass reduced-precision PE
mode: bf16-class throughput at N>=256, ~1.5e-4 matmul rel err).
"""

import numpy as np

import concourse.bass as bass  # noqa: F401  (bass types via bacc/tile)
import concourse.mybir as mybir
import concourse.tile as tile
from concourse import bacc
from concourse.bass_utils import run_bass_kernel_spmd

N_CORES = 8
IN_C = 512
OUT_C = 1024
N = 8192
NS = N // N_CORES  # columns per core
NCH = 512  # psum free-dim chunk (one bank)
N_CHUNKS = NS // NCH
KX = IN_C // 128  # k-tiles of the input projection
KH = OUT_C // 128  # k-tiles of the hidden projection
M_TILES = OUT_C // 128

F32 = mybir.dt.float32
MM_MODE = "fp16"  # one of: "fp32r", "bf16", "fp16"
import ml_dtypes as _mld
MM_DT = {"fp32r": mybir.dt.float32r, "bf16": mybir.dt.bfloat16,
         "fp16": mybir.dt.float16}[MM_MODE]
MM_NP = {"fp32r": np.float32, "bf16": _mld.bfloat16,
         "fp16": np.float16}[MM_MODE]

SIG = mybir.ActivationFunctionType.Sigmoid
TANH = mybir.ActivationFunctionType.Tanh
MULT = mybir.AluOpType.mult
ADD = mybir.AluOpType.add
COPY = mybir.ActivationFunctionType.Copy


def _build(ws0: float, ws1: float):
    nc = bacc.Bacc(
        "TRN2", target_bir_lowering=False, debug=False, num_devices=N_CORES
    )

    xd = nc.dram_tensor("x", [128, KX, NS], MM_DT, kind="ExternalInput")
    hd_ = [
        nc.dram_tensor(f"h{d}", [128, KH, NS], MM_DT, kind="ExternalInput")
        for d in (0, 1)
    ]
    cd_ = [
        nc.dram_tensor(f"c{d}", [OUT_C, NS], F32, kind="ExternalInput")
        for d in (0, 1)
    ]
    # weights: [gate, m_tile, partition(k%128), k_tile, m_in_tile]
    wxd = nc.dram_tensor("wx", [4, M_TILES, 128, KX, 128], MM_DT, kind="ExternalInput")
    whd = nc.dram_tensor("wh", [4, M_TILES, 128, KH, 128], MM_DT, kind="ExternalInput")
    biasd = nc.dram_tensor("bias", [128, 4 * M_TILES], F32, kind="ExternalInput")
    ctd = nc.dram_tensor("ct", [OUT_C, NS], F32, kind="ExternalOutput")
    htd = nc.dram_tensor("ht", [OUT_C, NS], F32, kind="ExternalOutput")

    with tile.TileContext(nc) as tc:
        with (
            tc.tile_pool(name="resident", bufs=1) as res_pool,
            tc.tile_pool(name="wx", bufs=8) as wx_pool,
            tc.tile_pool(name="wh", bufs=8) as wh_pool,
            tc.tile_pool(name="psum", bufs=8, space="PSUM") as ps_pool,
            tc.tile_pool(name="xproj", bufs=20) as xp_pool,
            tc.tile_pool(name="gates", bufs=6) as g_pool,
            tc.tile_pool(name="cprev", bufs=3) as cp_pool,
            tc.tile_pool(name="tmp", bufs=2) as t_pool,
            tc.tile_pool(name="dirres", bufs=4) as dr_pool,
            tc.tile_pool(name="out", bufs=2) as o_pool,
        ):
            wx_tiles: dict = {}
            wh_tiles: dict = {}

            def alloc_w(mt):
                wx_tiles[mt] = [
                    wx_pool.tile([128, KX, 128], MM_DT, tag="wx", name=f"wx_{mt}_{g}")
                    for g in range(4)
                ]
                wh_tiles[mt] = [
                    wh_pool.tile([128, KH, 128], MM_DT, tag="wh", name=f"wh_{mt}_{g}")
                    for g in range(4)
                ]

            def load_w(mt):
                alloc_w(mt)
                for g in range(4):
                    nc.sync.dma_start(wx_tiles[mt][g][:], wxd[g, mt])
                    nc.sync.dma_start(wh_tiles[mt][g][:], whd[g, mt])

            x_sb = res_pool.tile([128, KX, NS], MM_DT, tag="x")
            h_sb = [
                res_pool.tile([128, KH, NS], MM_DT, tag=f"h{d}", name=f"h_sb{d}")
                for d in (0, 1)
            ]
            bias_sb = res_pool.tile([128, 4 * M_TILES], F32, tag="bias")

            # Startup: order DMAs by first use (wx0+x-n0 gate the first
            # matmuls; wh0+h-n0 are needed ~13us later by the first dir
            # phase). Meanwhile ~5us of throwaway fp32 matmuls on the bias
            # tile warm the PE HAM clock gate (idle default is 1.2GHz; it
            # takes ~3.4us of busy PE to unthrottle to 2.4GHz) so the real
            # stream starts warm.
            nc.sync.dma_start(bias_sb[:], biasd[:])
            alloc_w(0)
            for g in range(4):
                nc.sync.dma_start(wx_tiles[0][g][:], wxd[g, 0])
            nsl0 = slice(0, NCH)
            nc.sync.dma_start(x_sb[:, :, nsl0], xd[:, :, nsl0])

            warm_ps = ps_pool.tile([128, NCH], F32, tag="ps", name="warm_ps")
            N_WARM = 48
            for i in range(N_WARM):
                nc.tensor.matmul(
                    warm_ps[:32, :32],
                    bias_sb[:, :32],
                    bias_sb[:, :32],
                    start=(i == 0),
                    stop=(i == N_WARM - 1),
                )

            for g in range(4):
                nc.sync.dma_start(wh_tiles[0][g][:], whd[g, 0])
            nc.sync.dma_start(h_sb[0][:, :, nsl0], hd_[0][:, :, nsl0])
            nc.sync.dma_start(h_sb[1][:, :, nsl0], hd_[1][:, :, nsl0])
            nsl1 = slice(NCH, 2 * NCH)
            nc.sync.dma_start(x_sb[:, :, nsl1], xd[:, :, nsl1])
            nc.sync.dma_start(h_sb[0][:, :, nsl1], hd_[0][:, :, nsl1])
            nc.sync.dma_start(h_sb[1][:, :, nsl1], hd_[1][:, :, nsl1])
            load_w(1)

            # Compute chunking: 512-wide PSUM-bank chunks, except the last
            # m-tile which ends with two 256-wide chunks so the post-matmul
            # tail (gate acts + cell update + output DMA after the final MM)
            # is half as long.
            def chunks_of(mt):
                return [(0, NCH), (NCH, NCH)]

            def px_phase(mt, ci, off, w, wxm):
                nsl = slice(off, off + w)
                xp = []
                for g in range(4):
                    px = ps_pool.tile(
                        [128, w], F32, tag="ps", name=f"px_{mt}_{ci}_{g}"
                    )
                    for kt in range(KX):
                        nc.tensor.matmul(
                            px[:],
                            wxm[g][:, kt, :],
                            x_sb[:, kt, nsl],
                            start=(kt == 0),
                            stop=(kt == KX - 1),
                        )
                    xpt = xp_pool.tile(
                        [128, w], F32, tag="xp", name=f"xp_{mt}_{ci}_{g}"
                    )
                    nc.scalar.activation(xpt[:], px[:], COPY)
                    xp.append(xpt)
                return xp

            def dir_phase(mt, ci, off, w, d, xp, whm, msl):
                nsl = slice(off, off + w)
                gt = []
                for g in range(4):
                    ps = ps_pool.tile(
                        [128, w], F32, tag="ps", name=f"ps_{mt}_{ci}_{d}_{g}"
                    )
                    # inject the shared x-projection, then accumulate the
                    # hidden projection on top of it
                    nc.vector.tensor_copy(ps[:], xp[g][:])
                    for kh in range(KH):
                        nc.tensor.matmul(
                            ps[:],
                            whm[g][:, kh, :],
                            h_sb[d][:, kh, nsl],
                            start=False,
                            stop=(kh == KH - 1),
                            skip_group_check=True,
                        )
                    gact = g_pool.tile(
                        [128, w], F32, tag="gate", name=f"gate_{mt}_{ci}_{d}_{g}"
                    )
                    nc.scalar.activation(
                        gact[:],
                        ps[:],
                        TANH if g == 2 else SIG,
                        bias=bias_sb[:, g * M_TILES + mt : g * M_TILES + mt + 1],
                    )
                    gt.append(gact)

                cp = cp_pool.tile([128, w], F32, tag="cp")
                nc.sync.dma_start(cp[:], cd_[d][msl, nsl])
                ig = t_pool.tile([128, w], F32, tag="ig")
                nc.vector.tensor_mul(ig[:], gt[0][:], gt[2][:])
                fc = t_pool.tile([128, w], F32, tag="fc")
                nc.vector.tensor_mul(fc[:], gt[1][:], cp[:])
                cnew = dr_pool.tile([128, w], F32, tag="cnew")
                nc.vector.tensor_add(cnew[:], ig[:], fc[:])
                tch = t_pool.tile([128, w], F32, tag="tch")
                nc.scalar.activation(tch[:], cnew[:], TANH)
                hnew = dr_pool.tile([128, w], F32, tag="hnew")
                nc.vector.tensor_mul(hnew[:], gt[3][:], tch[:])
                return cnew, hnew

            def combine(off, w, msl, cdir, hdir):
                nsl = slice(off, off + w)
                c0s = t_pool.tile([128, w], F32, tag="c0s")
                nc.vector.tensor_scalar_mul(c0s[:], cdir[0][:], ws0)
                ctt = o_pool.tile([128, w], F32, tag="ctt")
                nc.vector.scalar_tensor_tensor(
                    ctt[:], cdir[1][:], ws1, c0s[:], MULT, ADD
                )
                nc.sync.dma_start(ctd[msl, nsl], ctt[:])
                h0s = t_pool.tile([128, w], F32, tag="h0s")
                nc.vector.tensor_scalar_mul(h0s[:], hdir[0][:], ws0)
                htt = o_pool.tile([128, w], F32, tag="htt")
                nc.vector.scalar_tensor_tensor(
                    htt[:], hdir[1][:], ws1, h0s[:], MULT, ADD
                )
                nc.sync.dma_start(htd[msl, nsl], htt[:])

            # Software-pipelined x-projections: px(mt+1) is issued before
            # dirs(mt) so the PE has ~7us of weight/x-only work to chew on
            # whenever the hidden-projection inputs (wh, h, at startup) or
            # PSUM banks lag. At kernel start px(0)+px(1) = 64 MMs cover the
            # wh0/h0 DMA window that previously left a ~9us PE gap. These
            # early start=True groups also cover all 8 PSUM banks before any
            # start=False inject group runs (defined has_written state).
            xp_store: dict = {}

            def issue_px(mt):
                wxm = wx_tiles.pop(mt)
                for ci, (off, w) in enumerate(chunks_of(mt)):
                    xp_store[(mt, ci)] = px_phase(mt, ci, off, w, wxm)

            issue_px(0)
            for mt in range(M_TILES):
                msl = slice(mt * 128, (mt + 1) * 128)
                if mt + 2 < M_TILES:
                    load_w(mt + 2)
                whm = wh_tiles.pop(mt)
                if mt + 1 < M_TILES:
                    issue_px(mt + 1)

                for ci, (off, w) in enumerate(chunks_of(mt)):
                    xps = xp_store.pop((mt, ci))
                    c0, h0 = dir_phase(mt, ci, off, w, 0, xps, whm, msl)
                    c1, h1 = dir_phase(mt, ci, off, w, 1, xps, whm, msl)
                    combine(off, w, msl, [c0, c1], [h0, h1])

    nc.finalize()
    n_mm = sum(
        1 for i in nc.inst_map.values() if type(i).__name__ == "InstMatmult"
    )
    expected_mm = 2 * M_TILES * 4 * (KX + 2 * KH) + 48
    assert n_mm == expected_mm, f"matmul count {n_mm} != {expected_mm}"
    return nc


_CACHE: dict = {}


def _get_nc(ws0: float, ws1: float):
    key = (ws0, ws1)
    if key not in _CACHE:
        _CACHE.clear()
        _CACHE[key] = _build(ws0, ws1)
    return _CACHE[key]


def _prep_w(w: np.ndarray, kt: int) -> np.ndarray:
    """(OUT_C, K) weight -> [m_tile, partition, k_tile, m_in_tile] lhsT tiles."""
    wT = np.ascontiguousarray(w.T)  # (K, OUT_C)
    k = wT.shape[0]
    assert k == kt * 128
    r = wT.reshape(kt, 128, M_TILES, 128)  # [ktile, p, mtile, mi]
    return np.ascontiguousarray(r.transpose(2, 1, 0, 3).astype(MM_NP))


def _prep_rhs(a: np.ndarray, kt: int) -> np.ndarray:
    """(K, n) activation -> [partition, k_tile, n]."""
    k, n = a.shape
    assert k == kt * 128
    return np.ascontiguousarray(a.reshape(kt, 128, n).transpose(1, 0, 2).astype(MM_NP))


def run(inputs: dict, trace: bool = False, trace_kwargs: dict | None = None):
    x = np.asarray(inputs["x"], dtype=np.float32)
    ws = np.asarray(inputs["weighted_sum"], dtype=np.float32)
    ws0, ws1 = float(ws[0]), float(ws[1])
    nc = _get_nc(ws0, ws1)

    wx_host = np.stack(
        [_prep_w(np.asarray(inputs[k], dtype=np.float32), KX)
         for k in ("w_ii", "w_if", "w_ig", "w_io")]
    )
    wh_host = np.stack(
        [_prep_w(np.asarray(inputs[k], dtype=np.float32), KH)
         for k in ("w_hi", "w_hf", "w_hg", "w_ho")]
    )
    bias_host = np.concatenate(
        [np.asarray(inputs[k], dtype=np.float32).reshape(M_TILES, 128).T
         for k in ("b_i", "b_f", "b_g", "b_o")],
        axis=1,
    )
    bias_host = np.ascontiguousarray(bias_host)

    h0 = np.asarray(inputs["h_prev_dim0"], dtype=np.float32)
    h1 = np.asarray(inputs["h_prev_dim1"], dtype=np.float32)
    c0 = np.asarray(inputs["c_prev_dim0"], dtype=np.float32)
    c1 = np.asarray(inputs["c_prev_dim1"], dtype=np.float32)

    in_maps = []
    for core in range(N_CORES):
        csl = slice(core * NS, (core + 1) * NS)
        in_maps.append(
            {
                "x": _prep_rhs(x[:, csl], KX),
                "h0": _prep_rhs(h0[:, csl], KH),
                "h1": _prep_rhs(h1[:, csl], KH),
                "c0": np.ascontiguousarray(c0[:, csl]),
                "c1": np.ascontiguousarray(c1[:, csl]),
                "wx": wx_host,
                "wh": wh_host,
                "bias": bias_host,
            }
        )

    res = run_bass_kernel_spmd(
        nc,
        in_maps,
        list(range(N_CORES)),
        trace=trace,
        **(trace_kwargs or {}),
    )
    ct = np.concatenate([res.results[c]["ct"] for c in range(N_CORES)], axis=1)
    ht = np.concatenate([res.results[c]["ht"] for c in range(N_CORES)], axis=1)
    return (ct, ht), res


def kernel(**inputs) -> tuple:
    (ct, ht), _ = run(inputs)
    return ct, ht



# revision 11
# speedup vs baseline: 1.0465x; 1.0465x over previous
"""MDLSTM cell (2-direction) Bass/Tile kernel for Trainium2, 8-core SPMD.

Math (per direction d, with shared input projections):
    i = sigmoid(w_ii @ x + w_hi @ h_d + b_i)
    f = sigmoid(w_if @ x + w_hf @ h_d + b_f)
    g = tanh   (w_ig @ x + w_hg @ h_d + b_g)
    o = sigmoid(w_io @ x + w_ho @ h_d + b_o)
    c_d = f * c_prev_d + i * g
    h_d = o * tanh(c_d)
ct = ws0 * c_0 + ws1 * c_1 ;  ht = ws0 * h_0 + ws1 * h_1

Sharding: all activations/states split along N (=8192) across 8 cores;
weights replicated. No cross-core communication.

Per-core kernel: per output row tile (M=128) the 4 shared input
projections are computed once into PSUM (start=True groups) and copied to
SBUF; each of the 8 gate/direction accumulations then starts by injecting
that x-projection into its PSUM bank via a VectorE copy and accumulates
the 8 hidden-projection K-tiles on top (start=False matmuls — PE-write
accumulate onto engine-written PSUM, valid because every bank's first
group in program order is a start=True group that defines has_written).
ScalarE applies sigmoid/tanh + per-partition bias straight out of PSUM;
VectorE does the elementwise cell update and direction combine. Matmul
operands use float32r (fp32 storage, single-pass reduced-precision PE
mode: bf16-class throughput at N>=256, ~1.5e-4 matmul rel err).
"""

import numpy as np

import concourse.bass as bass  # noqa: F401  (bass types via bacc/tile)
import concourse.mybir as mybir
import concourse.tile as tile
from concourse import bacc
from concourse.bass_utils import run_bass_kernel_spmd

N_CORES = 8
IN_C = 512
OUT_C = 1024
N = 8192
NS = N // N_CORES  # columns per core
NCH = 512  # psum free-dim chunk (one bank)
N_CHUNKS = NS // NCH
KX = IN_C // 128  # k-tiles of the input projection
KH = OUT_C // 128  # k-tiles of the hidden projection
M_TILES = OUT_C // 128

F32 = mybir.dt.float32
MM_MODE = "fp16"  # one of: "fp32r", "bf16", "fp16"
import ml_dtypes as _mld
MM_DT = {"fp32r": mybir.dt.float32r, "bf16": mybir.dt.bfloat16,
         "fp16": mybir.dt.float16}[MM_MODE]
MM_NP = {"fp32r": np.float32, "bf16": _mld.bfloat16,
         "fp16": np.float16}[MM_MODE]

SIG = mybir.ActivationFunctionType.Sigmoid
TANH = mybir.ActivationFunctionType.Tanh
MULT = mybir.AluOpType.mult
ADD = mybir.AluOpType.add
COPY = mybir.ActivationFunctionType.Copy


def _build(ws0: float, ws1: float):
    nc = bacc.Bacc(
        "TRN2", target_bir_lowering=False, debug=False, num_devices=N_CORES
    )

    xd = nc.dram_tensor("x", [128, KX, NS], MM_DT, kind="ExternalInput")
    hd_ = [
        nc.dram_tensor(f"h{d}", [128, KH, NS], MM_DT, kind="ExternalInput")
        for d in (0, 1)
    ]
    cd_ = [
        nc.dram_tensor(f"c{d}", [OUT_C, NS], F32, kind="ExternalInput")
        for d in (0, 1)
    ]
    # weights: [gate, m_tile, partition(k%128), k_tile, m_in_tile]
    wxd = nc.dram_tensor("wx", [4, M_TILES, 128, KX, 128], MM_DT, kind="ExternalInput")
    whd = nc.dram_tensor("wh", [4, M_TILES, 128, KH, 128], MM_DT, kind="ExternalInput")
    biasd = nc.dram_tensor("bias", [128, 4 * M_TILES], F32, kind="ExternalInput")
    ctd = nc.dram_tensor("ct", [OUT_C, NS], F32, kind="ExternalOutput")
    htd = nc.dram_tensor("ht", [OUT_C, NS], F32, kind="ExternalOutput")

    with tile.TileContext(nc) as tc:
        with (
            tc.tile_pool(name="resident", bufs=1) as res_pool,
            tc.tile_pool(name="wx", bufs=8) as wx_pool,
            tc.tile_pool(name="wh", bufs=8) as wh_pool,
            tc.tile_pool(name="psum", bufs=8, space="PSUM") as ps_pool,
            tc.tile_pool(name="xproj", bufs=20) as xp_pool,
            tc.tile_pool(name="gates", bufs=6) as g_pool,
            tc.tile_pool(name="cprev", bufs=3) as cp_pool,
            tc.tile_pool(name="tmp", bufs=2) as t_pool,
            tc.tile_pool(name="dirres", bufs=4) as dr_pool,
            tc.tile_pool(name="out", bufs=2) as o_pool,
        ):
            wx_tiles: dict = {}
            wh_tiles: dict = {}

            def load_wx(mt):
                wx_tiles[mt] = [
                    wx_pool.tile([128, KX, 128], MM_DT, tag="wx", name=f"wx_{mt}_{g}")
                    for g in range(4)
                ]
                for g in range(4):
                    nc.sync.dma_start(wx_tiles[mt][g][:], wxd[g, mt])

            def load_wh(mt, gates=range(4)):
                if mt not in wh_tiles:
                    wh_tiles[mt] = [
                        wh_pool.tile(
                            [128, KH, 128], MM_DT, tag="wh", name=f"wh_{mt}_{g}"
                        )
                        for g in range(4)
                    ]
                for g in gates:
                    nc.sync.dma_start(wh_tiles[mt][g][:], whd[g, mt])

            def load_w(mt):
                load_wx(mt)
                load_wh(mt)

            x_sb = res_pool.tile([128, KX, NS], MM_DT, tag="x")
            h_sb = [
                res_pool.tile([128, KH, NS], MM_DT, tag=f"h{d}", name=f"h_sb{d}")
                for d in (0, 1)
            ]
            bias_sb = res_pool.tile([128, 4 * M_TILES], F32, tag="bias")

            # Startup: DMAs strictly in first-use order. The PE's early work
            # queue is px(0) then px(1) (x-projections, needing only wx+x);
            # the first hidden-projection group additionally needs wh0[g0] +
            # h0-n0. Meanwhile ~5us of throwaway fp32 matmuls on the bias
            # tile warm the PE HAM clock gate (idle default is 1.2GHz; it
            # takes ~3.4us of busy PE to unthrottle to 2.4GHz) so the real
            # stream starts warm.
            nc.sync.dma_start(bias_sb[:], biasd[:])
            load_wx(0)
            nsl0 = slice(0, NCH)
            nsl1 = slice(NCH, 2 * NCH)
            nc.sync.dma_start(x_sb[:, :, nsl0], xd[:, :, nsl0])

            warm_ps = ps_pool.tile([128, NCH], F32, tag="ps", name="warm_ps")
            N_WARM = 48
            for i in range(N_WARM):
                nc.tensor.matmul(
                    warm_ps[:32, :32],
                    bias_sb[:, :32],
                    bias_sb[:, :32],
                    start=(i == 0),
                    stop=(i == N_WARM - 1),
                )

            nc.sync.dma_start(x_sb[:, :, nsl1], xd[:, :, nsl1])
            load_wx(1)
            load_wh(0, gates=[0])
            nc.sync.dma_start(h_sb[0][:, :, nsl0], hd_[0][:, :, nsl0])
            load_wh(0, gates=[1, 2, 3])
            nc.sync.dma_start(h_sb[1][:, :, nsl0], hd_[1][:, :, nsl0])
            nc.sync.dma_start(h_sb[0][:, :, nsl1], hd_[0][:, :, nsl1])
            nc.sync.dma_start(h_sb[1][:, :, nsl1], hd_[1][:, :, nsl1])
            load_wh(1)

            # Compute chunking: 512-wide PSUM-bank chunks, except the last
            # m-tile which ends with two 256-wide chunks so the post-matmul
            # tail (gate acts + cell update + output DMA after the final MM)
            # is half as long.
            def chunks_of(mt):
                return [(0, NCH), (NCH, NCH)]

            def px_phase(mt, ci, off, w, wxm):
                nsl = slice(off, off + w)
                xp = []
                for g in range(4):
                    px = ps_pool.tile(
                        [128, w], F32, tag="ps", name=f"px_{mt}_{ci}_{g}"
                    )
                    for kt in range(KX):
                        nc.tensor.matmul(
                            px[:],
                            wxm[g][:, kt, :],
                            x_sb[:, kt, nsl],
                            start=(kt == 0),
                            stop=(kt == KX - 1),
                        )
                    xpt = xp_pool.tile(
                        [128, w], F32, tag="xp", name=f"xp_{mt}_{ci}_{g}"
                    )
                    nc.scalar.activation(xpt[:], px[:], COPY)
                    xp.append(xpt)
                return xp

            def dir_phase(mt, ci, off, w, d, xp, whm, msl):
                nsl = slice(off, off + w)
                gt = []
                for g in range(4):
                    ps = ps_pool.tile(
                        [128, w], F32, tag="ps", name=f"ps_{mt}_{ci}_{d}_{g}"
                    )
                    # inject the shared x-projection, then accumulate the
                    # hidden projection on top of it
                    nc.vector.tensor_copy(ps[:], xp[g][:])
                    for kh in range(KH):
                        nc.tensor.matmul(
                            ps[:],
                            whm[g][:, kh, :],
                            h_sb[d][:, kh, nsl],
                            start=False,
                            stop=(kh == KH - 1),
                            skip_group_check=True,
                        )
                    gact = g_pool.tile(
                        [128, w], F32, tag="gate", name=f"gate_{mt}_{ci}_{d}_{g}"
                    )
                    nc.scalar.activation(
                        gact[:],
                        ps[:],
                        TANH if g == 2 else SIG,
                        bias=bias_sb[:, g * M_TILES + mt : g * M_TILES + mt + 1],
                    )
                    gt.append(gact)

                cp = cp_pool.tile([128, w], F32, tag="cp")
                nc.sync.dma_start(cp[:], cd_[d][msl, nsl])
                ig = t_pool.tile([128, w], F32, tag="ig")
                nc.vector.tensor_mul(ig[:], gt[0][:], gt[2][:])
                fc = t_pool.tile([128, w], F32, tag="fc")
                nc.vector.tensor_mul(fc[:], gt[1][:], cp[:])
                cnew = dr_pool.tile([128, w], F32, tag="cnew")
                nc.vector.tensor_add(cnew[:], ig[:], fc[:])
                tch = t_pool.tile([128, w], F32, tag="tch")
                nc.scalar.activation(tch[:], cnew[:], TANH)
                hnew = dr_pool.tile([128, w], F32, tag="hnew")
                nc.vector.tensor_mul(hnew[:], gt[3][:], tch[:])
                return cnew, hnew

            def combine(off, w, msl, cdir, hdir):
                nsl = slice(off, off + w)
                c0s = t_pool.tile([128, w], F32, tag="c0s")
                nc.vector.tensor_scalar_mul(c0s[:], cdir[0][:], ws0)
                ctt = o_pool.tile([128, w], F32, tag="ctt")
                nc.vector.scalar_tensor_tensor(
                    ctt[:], cdir[1][:], ws1, c0s[:], MULT, ADD
                )
                nc.sync.dma_start(ctd[msl, nsl], ctt[:])
                h0s = t_pool.tile([128, w], F32, tag="h0s")
                nc.vector.tensor_scalar_mul(h0s[:], hdir[0][:], ws0)
                htt = o_pool.tile([128, w], F32, tag="htt")
                nc.vector.scalar_tensor_tensor(
                    htt[:], hdir[1][:], ws1, h0s[:], MULT, ADD
                )
                nc.sync.dma_start(htd[msl, nsl], htt[:])

            # Software-pipelined x-projections: px(mt+1) is issued before
            # dirs(mt) so the PE has ~7us of weight/x-only work to chew on
            # whenever the hidden-projection inputs (wh, h, at startup) or
            # PSUM banks lag. At kernel start px(0)+px(1) = 64 MMs cover the
            # wh0/h0 DMA window that previously left a ~9us PE gap. These
            # early start=True groups also cover all 8 PSUM banks before any
            # start=False inject group runs (defined has_written state).
            xp_store: dict = {}

            def issue_px(mt):
                wxm = wx_tiles.pop(mt)
                for ci, (off, w) in enumerate(chunks_of(mt)):
                    xp_store[(mt, ci)] = px_phase(mt, ci, off, w, wxm)

            issue_px(0)
            for mt in range(M_TILES):
                msl = slice(mt * 128, (mt + 1) * 128)
                if mt + 2 < M_TILES:
                    load_w(mt + 2)
                whm = wh_tiles.pop(mt)
                if mt + 1 < M_TILES:
                    issue_px(mt + 1)

                for ci, (off, w) in enumerate(chunks_of(mt)):
                    xps = xp_store.pop((mt, ci))
                    c0, h0 = dir_phase(mt, ci, off, w, 0, xps, whm, msl)
                    c1, h1 = dir_phase(mt, ci, off, w, 1, xps, whm, msl)
                    combine(off, w, msl, [c0, c1], [h0, h1])

    nc.finalize()
    n_mm = sum(
        1 for i in nc.inst_map.values() if type(i).__name__ == "InstMatmult"
    )
    expected_mm = 2 * M_TILES * 4 * (KX + 2 * KH) + 48
    assert n_mm == expected_mm, f"matmul count {n_mm} != {expected_mm}"
    return nc


_CACHE: dict = {}


def _get_nc(ws0: float, ws1: float):
    key = (ws0, ws1)
    if key not in _CACHE:
        _CACHE.clear()
        _CACHE[key] = _build(ws0, ws1)
    return _CACHE[key]


def _prep_w(w: np.ndarray, kt: int) -> np.ndarray:
    """(OUT_C, K) weight -> [m_tile, partition, k_tile, m_in_tile] lhsT tiles."""
    wT = np.ascontiguousarray(w.T)  # (K, OUT_C)
    k = wT.shape[0]
    assert k == kt * 128
    r = wT.reshape(kt, 128, M_TILES, 128)  # [ktile, p, mtile, mi]
    return np.ascontiguousarray(r.transpose(2, 1, 0, 3).astype(MM_NP))


def _prep_rhs(a: np.ndarray, kt: int) -> np.ndarray:
    """(K, n) activation -> [partition, k_tile, n]."""
    k, n = a.shape
    assert k == kt * 128
    return np.ascontiguousarray(a.reshape(kt, 128, n).transpose(1, 0, 2).astype(MM_NP))


def run(inputs: dict, trace: bool = False, trace_kwargs: dict | None = None):
    x = np.asarray(inputs["x"], dtype=np.float32)
    ws = np.asarray(inputs["weighted_sum"], dtype=np.float32)
    ws0, ws1 = float(ws[0]), float(ws[1])
    nc = _get_nc(ws0, ws1)

    wx_host = np.stack(
        [_prep_w(np.asarray(inputs[k], dtype=np.float32), KX)
         for k in ("w_ii", "w_if", "w_ig", "w_io")]
    )
    wh_host = np.stack(
        [_prep_w(np.asarray(inputs[k], dtype=np.float32), KH)
         for k in ("w_hi", "w_hf", "w_hg", "w_ho")]
    )
    bias_host = np.concatenate(
        [np.asarray(inputs[k], dtype=np.float32).reshape(M_TILES, 128).T
         for k in ("b_i", "b_f", "b_g", "b_o")],
        axis=1,
    )
    bias_host = np.ascontiguousarray(bias_host)

    h0 = np.asarray(inputs["h_prev_dim0"], dtype=np.float32)
    h1 = np.asarray(inputs["h_prev_dim1"], dtype=np.float32)
    c0 = np.asarray(inputs["c_prev_dim0"], dtype=np.float32)
    c1 = np.asarray(inputs["c_prev_dim1"], dtype=np.float32)

    in_maps = []
    for core in range(N_CORES):
        csl = slice(core * NS, (core + 1) * NS)
        in_maps.append(
            {
                "x": _prep_rhs(x[:, csl], KX),
                "h0": _prep_rhs(h0[:, csl], KH),
                "h1": _prep_rhs(h1[:, csl], KH),
                "c0": np.ascontiguousarray(c0[:, csl]),
                "c1": np.ascontiguousarray(c1[:, csl]),
                "wx": wx_host,
                "wh": wh_host,
                "bias": bias_host,
            }
        )

    res = run_bass_kernel_spmd(
        nc,
        in_maps,
        list(range(N_CORES)),
        trace=trace,
        **(trace_kwargs or {}),
    )
    ct = np.concatenate([res.results[c]["ct"] for c in range(N_CORES)], axis=1)
    ht = np.concatenate([res.results[c]["ht"] for c in range(N_CORES)], axis=1)
    return (ct, ht), res


def kernel(**inputs) -> tuple:
    (ct, ht), _ = run(inputs)
    return ct, ht



# revision 24
# speedup vs baseline: 1.1419x; 1.0912x over previous
"""MDLSTM cell (2-direction) Bass/Tile kernel for Trainium2, 8-core SPMD.

Math (per direction d, with shared input projections):
    i = sigmoid(w_ii @ x + w_hi @ h_d + b_i)
    f = sigmoid(w_if @ x + w_hf @ h_d + b_f)
    g = tanh   (w_ig @ x + w_hg @ h_d + b_g)
    o = sigmoid(w_io @ x + w_ho @ h_d + b_o)
    c_d = f * c_prev_d + i * g
    h_d = o * tanh(c_d)
ct = ws0 * c_0 + ws1 * c_1 ;  ht = ws0 * h_0 + ws1 * h_1

Sharding: all activations/states split along N (=8192) across 8 cores;
weights replicated. No cross-core communication.

Per-core kernel: per output row tile (M=128) the 4 shared input
projections are computed once into PSUM (start=True groups) and copied to
SBUF; each of the 8 gate/direction accumulations then starts by injecting
that x-projection into its PSUM bank via a VectorE copy and accumulates
the 8 hidden-projection K-tiles on top (start=False matmuls — PE-write
accumulate onto engine-written PSUM, valid because every bank's first
group in program order is a start=True group that defines has_written).
ScalarE applies sigmoid/tanh + per-partition bias straight out of PSUM;
VectorE does the elementwise cell update and direction combine. Matmul
operands use float32r (fp32 storage, single-pass reduced-precision PE
mode: bf16-class throughput at N>=256, ~1.5e-4 matmul rel err).
"""

import numpy as np

import concourse.bass as bass  # noqa: F401  (bass types via bacc/tile)
import concourse.mybir as mybir
import concourse.tile as tile
from concourse import bacc
from concourse.bass_utils import run_bass_kernel_spmd

N_CORES = 8
IN_C = 512
OUT_C = 1024
N = 8192
NS = N // N_CORES  # columns per core
NCH = 512  # psum free-dim chunk (one bank)
N_CHUNKS = NS // NCH
KX = IN_C // 128  # k-tiles of the input projection
KH = OUT_C // 128  # k-tiles of the hidden projection
M_TILES = OUT_C // 128
# Hidden-projection k-tiles 0..N_DR8-1 go through one fp8e4 DoubleRow
# matmul (2 k-tiles per pass, 2 MACs/cell/cycle); the rest stay fp16.
# Unscaled e4m3 (PSUM accumulation forbids per-matmul rescale); simulated
# end-to-end rel_fro = 1.37e-2 vs the 2e-2 gate. N_DR8 must be 0 or 2.
N_DR8 = 2
KHF = KH - N_DR8  # fp16 hidden k-tiles

F32 = mybir.dt.float32
MM_MODE = "fp16"  # one of: "fp32r", "bf16", "fp16"
import ml_dtypes as _mld
MM_DT = {"fp32r": mybir.dt.float32r, "bf16": mybir.dt.bfloat16,
         "fp16": mybir.dt.float16}[MM_MODE]
MM_NP = {"fp32r": np.float32, "bf16": _mld.bfloat16,
         "fp16": np.float16}[MM_MODE]

SIG = mybir.ActivationFunctionType.Sigmoid
TANH = mybir.ActivationFunctionType.Tanh
MULT = mybir.AluOpType.mult
ADD = mybir.AluOpType.add
COPY = mybir.ActivationFunctionType.Copy


def _build(ws0: float, ws1: float):
    nc = bacc.Bacc(
        "TRN2", target_bir_lowering=False, debug=False, num_devices=N_CORES
    )

    F8 = mybir.dt.float8e4
    xd = nc.dram_tensor("x", [128, KX, NS], MM_DT, kind="ExternalInput")
    hd_ = [
        nc.dram_tensor(f"h{d}", [128, KHF, NS], MM_DT, kind="ExternalInput")
        for d in (0, 1)
    ]
    h8d_ = [
        nc.dram_tensor(f"h8_{d}", [128, N_DR8, NS], F8, kind="ExternalInput")
        for d in (0, 1)
    ]
    cd_ = [
        nc.dram_tensor(f"c{d}", [OUT_C, NS], F32, kind="ExternalInput")
        for d in (0, 1)
    ]
    # weights: [gate, m_tile, partition(k%128), k_tile, m_in_tile]
    wxd = nc.dram_tensor("wx", [4, M_TILES, 128, KX, 128], MM_DT, kind="ExternalInput")
    whd = nc.dram_tensor("wh", [4, M_TILES, 128, KHF, 128], MM_DT, kind="ExternalInput")
    wh8d = nc.dram_tensor(
        "wh8", [4, M_TILES, 128, N_DR8, 128], F8, kind="ExternalInput"
    )
    biasd = nc.dram_tensor("bias", [128, 4 * M_TILES], F32, kind="ExternalInput")
    ctd = nc.dram_tensor("ct", [OUT_C, NS], F32, kind="ExternalOutput")
    htd = nc.dram_tensor("ht", [OUT_C, NS], F32, kind="ExternalOutput")

    with tile.TileContext(nc) as tc:
        with (
            tc.tile_pool(name="resident", bufs=1) as res_pool,
            tc.tile_pool(name="wx", bufs=8) as wx_pool,
            tc.tile_pool(name="wh", bufs=8) as wh_pool,
            tc.tile_pool(name="wh8", bufs=8) as wh8_pool,
            tc.tile_pool(name="psum", bufs=8, space="PSUM") as ps_pool,
            tc.tile_pool(name="xproj", bufs=20) as xp_pool,
            tc.tile_pool(name="gates", bufs=6) as g_pool,
            tc.tile_pool(name="cprev", bufs=3) as cp_pool,
            tc.tile_pool(name="tmp", bufs=2) as t_pool,
            tc.tile_pool(name="dirres", bufs=4) as dr_pool,
            tc.tile_pool(name="out", bufs=2) as o_pool,
        ):
            wx_tiles: dict = {}
            wh_tiles: dict = {}
            wh8_tiles: dict = {}

            def load_wx(mt):
                wx_tiles[mt] = [
                    wx_pool.tile([128, KX, 128], MM_DT, tag="wx", name=f"wx_{mt}_{g}")
                    for g in range(4)
                ]
                for g in range(4):
                    nc.sync.dma_start(wx_tiles[mt][g][:], wxd[g, mt])

            def load_wh(mt, gates=range(4)):
                if mt not in wh_tiles:
                    wh_tiles[mt] = [
                        wh_pool.tile(
                            [128, KHF, 128], MM_DT, tag="wh", name=f"wh_{mt}_{g}"
                        )
                        for g in range(4)
                    ]
                    wh8_tiles[mt] = [
                        wh8_pool.tile(
                            [128, N_DR8, 128], F8, tag="wh8", name=f"wh8_{mt}_{g}"
                        )
                        for g in range(4)
                    ]
                for g in gates:
                    nc.sync.dma_start(wh8_tiles[mt][g][:], wh8d[g, mt])
                    nc.sync.dma_start(wh_tiles[mt][g][:], whd[g, mt])

            def load_w(mt):
                load_wx(mt)
                load_wh(mt)

            x_sb = res_pool.tile([128, KX, NS], MM_DT, tag="x")
            h_sb = [
                res_pool.tile([128, KHF, NS], MM_DT, tag=f"h{d}", name=f"h_sb{d}")
                for d in (0, 1)
            ]
            h8_sb = [
                res_pool.tile([128, N_DR8, NS], F8, tag=f"h8_{d}", name=f"h8_sb{d}")
                for d in (0, 1)
            ]
            bias_sb = res_pool.tile([128, 4 * M_TILES], F32, tag="bias")

            # Startup: DMAs strictly in first-use order. The PE's early work
            # queue is px(0) then px(1) (x-projections, needing only wx+x);
            # the first hidden-projection group additionally needs wh0[g0] +
            # h0-n0. Meanwhile ~5us of throwaway fp32 matmuls on the bias
            # tile warm the PE HAM clock gate (idle default is 1.2GHz; it
            # takes ~3.4us of busy PE to unthrottle to 2.4GHz) so the real
            # stream starts warm.
            nc.sync.dma_start(bias_sb[:], biasd[:])
            load_wx(0)
            nsl0 = slice(0, NCH)
            nsl1 = slice(NCH, 2 * NCH)
            nc.sync.dma_start(x_sb[:, :, nsl0], xd[:, :, nsl0])

            warm_ps = ps_pool.tile([128, NCH], F32, tag="ps", name="warm_ps")
            N_WARM = 48
            for i in range(N_WARM):
                nc.tensor.matmul(
                    warm_ps[:32, :32],
                    bias_sb[:, :32],
                    bias_sb[:, :32],
                    start=(i == 0),
                    stop=(i == N_WARM - 1),
                )

            nc.sync.dma_start(x_sb[:, :, nsl1], xd[:, :, nsl1])
            load_wx(1)
            load_wh(0, gates=[0])
            nc.sync.dma_start(h8_sb[0][:, :, nsl0], h8d_[0][:, :, nsl0])
            nc.sync.dma_start(h_sb[0][:, :, nsl0], hd_[0][:, :, nsl0])
            load_wh(0, gates=[1, 2, 3])
            nc.sync.dma_start(h8_sb[1][:, :, nsl0], h8d_[1][:, :, nsl0])
            nc.sync.dma_start(h_sb[1][:, :, nsl0], hd_[1][:, :, nsl0])
            nc.sync.dma_start(h8_sb[0][:, :, nsl1], h8d_[0][:, :, nsl1])
            nc.sync.dma_start(h_sb[0][:, :, nsl1], hd_[0][:, :, nsl1])
            nc.sync.dma_start(h8_sb[1][:, :, nsl1], h8d_[1][:, :, nsl1])
            nc.sync.dma_start(h_sb[1][:, :, nsl1], hd_[1][:, :, nsl1])
            load_wh(1)

            # Compute chunking: 512-wide PSUM-bank chunks, except the last
            # m-tile which ends with two 256-wide chunks so the post-matmul
            # tail (gate acts + cell update + output DMA after the final MM)
            # is half as long.
            def chunks_of(mt):
                return [(0, NCH), (NCH, NCH)]

            def px_phase(mt, ci, off, w, wxm):
                nsl = slice(off, off + w)
                xp = []
                for g in range(4):
                    px = ps_pool.tile(
                        [128, w], F32, tag="ps", name=f"px_{mt}_{ci}_{g}"
                    )
                    for kt in range(KX):
                        nc.tensor.matmul(
                            px[:],
                            wxm[g][:, kt, :],
                            x_sb[:, kt, nsl],
                            start=(kt == 0),
                            stop=(kt == KX - 1),
                        )
                    xpt = xp_pool.tile(
                        [128, w], F32, tag="xp", name=f"xp_{mt}_{ci}_{g}"
                    )
                    nc.scalar.activation(xpt[:], px[:], COPY)
                    xp.append(xpt)
                return xp

            def dir_phase(mt, ci, off, w, d, xp, whm, wh8m, msl):
                nsl = slice(off, off + w)
                gt = []
                for g in range(4):
                    ps = ps_pool.tile(
                        [128, w], F32, tag="ps", name=f"ps_{mt}_{ci}_{d}_{g}"
                    )
                    # inject the shared x-projection, then accumulate the
                    # hidden projection on top of it: first k-tiles 0..1 as
                    # one fp8 DoubleRow pass, then k-tiles 2..7 in fp16
                    nc.vector.tensor_copy(ps[:], xp[g][:])
                    if N_DR8:
                        nc.tensor.matmul(
                            ps[:],
                            wh8m[g][:],
                            h8_sb[d][:, :, nsl],
                            start=False,
                            stop=False,
                            skip_group_check=True,
                            perf_mode=mybir.MatmulPerfMode.DoubleRow,
                        )
                    for kh in range(KHF):
                        nc.tensor.matmul(
                            ps[:],
                            whm[g][:, kh, :],
                            h_sb[d][:, kh, nsl],
                            start=False,
                            stop=(kh == KHF - 1),
                            skip_group_check=True,
                        )
                    gact = g_pool.tile(
                        [128, w], F32, tag="gate", name=f"gate_{mt}_{ci}_{d}_{g}"
                    )
                    nc.scalar.activation(
                        gact[:],
                        ps[:],
                        TANH if g == 2 else SIG,
                        bias=bias_sb[:, g * M_TILES + mt : g * M_TILES + mt + 1],
                    )
                    gt.append(gact)

                cp = cp_pool.tile([128, w], F32, tag="cp")
                nc.sync.dma_start(cp[:], cd_[d][msl, nsl])
                ig = t_pool.tile([128, w], F32, tag="ig")
                nc.vector.tensor_mul(ig[:], gt[0][:], gt[2][:])
                fc = t_pool.tile([128, w], F32, tag="fc")
                nc.vector.tensor_mul(fc[:], gt[1][:], cp[:])
                cnew = dr_pool.tile([128, w], F32, tag="cnew")
                nc.vector.tensor_add(cnew[:], ig[:], fc[:])
                tch = t_pool.tile([128, w], F32, tag="tch")
                nc.scalar.activation(tch[:], cnew[:], TANH)
                hnew = dr_pool.tile([128, w], F32, tag="hnew")
                nc.vector.tensor_mul(hnew[:], gt[3][:], tch[:])
                return cnew, hnew

            def combine(off, w, msl, cdir, hdir):
                nsl = slice(off, off + w)
                c0s = t_pool.tile([128, w], F32, tag="c0s")
                nc.vector.tensor_scalar_mul(c0s[:], cdir[0][:], ws0)
                ctt = o_pool.tile([128, w], F32, tag="ctt")
                nc.vector.scalar_tensor_tensor(
                    ctt[:], cdir[1][:], ws1, c0s[:], MULT, ADD
                )
                nc.sync.dma_start(ctd[msl, nsl], ctt[:])
                h0s = t_pool.tile([128, w], F32, tag="h0s")
                nc.vector.tensor_scalar_mul(h0s[:], hdir[0][:], ws0)
                htt = o_pool.tile([128, w], F32, tag="htt")
                nc.vector.scalar_tensor_tensor(
                    htt[:], hdir[1][:], ws1, h0s[:], MULT, ADD
                )
                nc.sync.dma_start(htd[msl, nsl], htt[:])

            # Software-pipelined x-projections: px(mt+1) is issued before
            # dirs(mt) so the PE has ~7us of weight/x-only work to chew on
            # whenever the hidden-projection inputs (wh, h, at startup) or
            # PSUM banks lag. At kernel start px(0)+px(1) = 64 MMs cover the
            # wh0/h0 DMA window that previously left a ~9us PE gap. These
            # early start=True groups also cover all 8 PSUM banks before any
            # start=False inject group runs (defined has_written state).
            xp_store: dict = {}

            def issue_px(mt):
                wxm = wx_tiles.pop(mt)
                for ci, (off, w) in enumerate(chunks_of(mt)):
                    xp_store[(mt, ci)] = px_phase(mt, ci, off, w, wxm)

            issue_px(0)
            for mt in range(M_TILES):
                msl = slice(mt * 128, (mt + 1) * 128)
                if mt + 2 < M_TILES:
                    load_w(mt + 2)
                whm = wh_tiles.pop(mt)
                wh8m = wh8_tiles.pop(mt)
                if mt + 1 < M_TILES:
                    issue_px(mt + 1)

                for ci, (off, w) in enumerate(chunks_of(mt)):
                    xps = xp_store.pop((mt, ci))
                    c0, h0 = dir_phase(mt, ci, off, w, 0, xps, whm, wh8m, msl)
                    c1, h1 = dir_phase(mt, ci, off, w, 1, xps, whm, wh8m, msl)
                    combine(off, w, msl, [c0, c1], [h0, h1])

    nc.finalize()
    n_mm = sum(
        1 for i in nc.inst_map.values() if type(i).__name__ == "InstMatmult"
    )
    expected_mm = 2 * M_TILES * 4 * (KX + 2 * (KHF + (1 if N_DR8 else 0))) + 48
    assert n_mm == expected_mm, f"matmul count {n_mm} != {expected_mm}"
    return nc


_CACHE: dict = {}


def _get_nc(ws0: float, ws1: float):
    key = (ws0, ws1)
    if key not in _CACHE:
        _CACHE.clear()
        _CACHE[key] = _build(ws0, ws1)
    return _CACHE[key]


F8_NP = _mld.float8_e4m3  # TRN fp8e4: IEEE e4m3, max normal +-240


def _prep_w(w: np.ndarray, kt: int, np_dt=None) -> np.ndarray:
    """(K, OUT_C)-transposed weight rows -> [m_tile, partition, k_tile, m_in_tile]."""
    wT = w
    k = wT.shape[0]
    assert k == kt * 128
    r = wT.reshape(kt, 128, M_TILES, 128)  # [ktile, p, mtile, mi]
    return np.ascontiguousarray(
        r.transpose(2, 1, 0, 3).astype(np_dt if np_dt is not None else MM_NP)
    )


def _prep_rhs(a: np.ndarray, kt: int, np_dt=None) -> np.ndarray:
    """(K, n) activation rows -> [partition, k_tile, n]."""
    k, n = a.shape
    assert k == kt * 128
    return np.ascontiguousarray(
        a.reshape(kt, 128, n).transpose(1, 0, 2).astype(
            np_dt if np_dt is not None else MM_NP
        )
    )


def run(inputs: dict, trace: bool = False, trace_kwargs: dict | None = None):
    x = np.asarray(inputs["x"], dtype=np.float32)
    ws = np.asarray(inputs["weighted_sum"], dtype=np.float32)
    ws0, ws1 = float(ws[0]), float(ws[1])
    nc = _get_nc(ws0, ws1)

    wx_host = np.stack(
        [_prep_w(np.ascontiguousarray(np.asarray(inputs[k], dtype=np.float32).T), KX)
         for k in ("w_ii", "w_if", "w_ig", "w_io")]
    )
    whT = [
        np.ascontiguousarray(np.asarray(inputs[k], dtype=np.float32).T)
        for k in ("w_hi", "w_hf", "w_hg", "w_ho")
    ]
    kc = N_DR8 * 128
    wh_host = np.stack([_prep_w(w[kc:], KHF) for w in whT])
    wh8_host = np.stack([_prep_w(w[:kc], N_DR8, F8_NP) for w in whT])
    bias_host = np.concatenate(
        [np.asarray(inputs[k], dtype=np.float32).reshape(M_TILES, 128).T
         for k in ("b_i", "b_f", "b_g", "b_o")],
        axis=1,
    )
    bias_host = np.ascontiguousarray(bias_host)

    h0 = np.asarray(inputs["h_prev_dim0"], dtype=np.float32)
    h1 = np.asarray(inputs["h_prev_dim1"], dtype=np.float32)
    c0 = np.asarray(inputs["c_prev_dim0"], dtype=np.float32)
    c1 = np.asarray(inputs["c_prev_dim1"], dtype=np.float32)

    in_maps = []
    for core in range(N_CORES):
        csl = slice(core * NS, (core + 1) * NS)
        in_maps.append(
            {
                "x": _prep_rhs(x[:, csl], KX),
                "h0": _prep_rhs(h0[kc:, csl], KHF),
                "h1": _prep_rhs(h1[kc:, csl], KHF),
                "h8_0": _prep_rhs(h0[:kc, csl], N_DR8, F8_NP),
                "h8_1": _prep_rhs(h1[:kc, csl], N_DR8, F8_NP),
                "c0": np.ascontiguousarray(c0[:, csl]),
                "c1": np.ascontiguousarray(c1[:, csl]),
                "wx": wx_host,
                "wh": wh_host,
                "wh8": wh8_host,
                "bias": bias_host,
            }
        )

    res = run_bass_kernel_spmd(
        nc,
        in_maps,
        list(range(N_CORES)),
        trace=trace,
        **(trace_kwargs or {}),
    )
    ct = np.concatenate([res.results[c]["ct"] for c in range(N_CORES)], axis=1)
    ht = np.concatenate([res.results[c]["ht"] for c in range(N_CORES)], axis=1)
    return (ct, ht), res


def kernel(**inputs) -> tuple:
    (ct, ht), _ = run(inputs)
    return ct, ht



# revision 33
# speedup vs baseline: 1.1443x; 1.0021x over previous
"""MDLSTM cell (2-direction) Bass/Tile kernel for Trainium2, 8-core SPMD.

Math (per direction d, with shared input projections):
    i = sigmoid(w_ii @ x + w_hi @ h_d + b_i)
    f = sigmoid(w_if @ x + w_hf @ h_d + b_f)
    g = tanh   (w_ig @ x + w_hg @ h_d + b_g)
    o = sigmoid(w_io @ x + w_ho @ h_d + b_o)
    c_d = f * c_prev_d + i * g
    h_d = o * tanh(c_d)
ct = ws0 * c_0 + ws1 * c_1 ;  ht = ws0 * h_0 + ws1 * h_1

Sharding: all activations/states split along N (=8192) across 8 cores;
weights replicated. No cross-core communication.

Per-core kernel: per output row tile (M=128) the 4 shared input
projections are computed once into PSUM (start=True groups) and copied to
SBUF; each of the 8 gate/direction accumulations then starts by injecting
that x-projection into its PSUM bank via a VectorE copy and accumulates
the 8 hidden-projection K-tiles on top (start=False matmuls — PE-write
accumulate onto engine-written PSUM, valid because every bank's first
group in program order is a start=True group that defines has_written).
ScalarE applies sigmoid/tanh + per-partition bias straight out of PSUM;
VectorE does the elementwise cell update and direction combine. Matmul
operands use float32r (fp32 storage, single-pass reduced-precision PE
mode: bf16-class throughput at N>=256, ~1.5e-4 matmul rel err).
"""

import numpy as np

import concourse.bass as bass  # noqa: F401  (bass types via bacc/tile)
import concourse.mybir as mybir
import concourse.tile as tile
from concourse import bacc
from concourse.bass_utils import run_bass_kernel_spmd

N_CORES = 8
IN_C = 512
OUT_C = 1024
N = 8192
NS = N // N_CORES  # columns per core
NCH = 512  # psum free-dim chunk (one bank)
N_CHUNKS = NS // NCH
KX = IN_C // 128  # k-tiles of the input projection
KH = OUT_C // 128  # k-tiles of the hidden projection
M_TILES = OUT_C // 128
# Hidden-projection k-tiles 0..N_DR8-1 go through one fp8e4 DoubleRow
# matmul (2 k-tiles per pass, 2 MACs/cell/cycle); the rest stay fp16.
# Unscaled e4m3 (PSUM accumulation forbids per-matmul rescale); simulated
# end-to-end rel_fro = 1.37e-2 vs the 2e-2 gate. N_DR8 must be 0 or 2.
N_DR8 = 2
KHF = KH - N_DR8  # fp16 hidden k-tiles

F32 = mybir.dt.float32
MM_MODE = "fp16"  # one of: "fp32r", "bf16", "fp16"
import ml_dtypes as _mld
MM_DT = {"fp32r": mybir.dt.float32r, "bf16": mybir.dt.bfloat16,
         "fp16": mybir.dt.float16}[MM_MODE]
MM_NP = {"fp32r": np.float32, "bf16": _mld.bfloat16,
         "fp16": np.float16}[MM_MODE]

SIG = mybir.ActivationFunctionType.Sigmoid
TANH = mybir.ActivationFunctionType.Tanh
MULT = mybir.AluOpType.mult
ADD = mybir.AluOpType.add
COPY = mybir.ActivationFunctionType.Copy


def _build(ws0: float, ws1: float):
    nc = bacc.Bacc(
        "TRN2", target_bir_lowering=False, debug=False, num_devices=N_CORES
    )

    F8 = mybir.dt.float8e4
    # Activations are stored per 512-column chunk, fully contiguous, so each
    # chunk is ONE dma_start: the Sync sequencer spends ~0.8us of serial time
    # per issued DMA and the startup critical path is issue-count-bound.
    xd_ = [
        nc.dram_tensor(f"x{n}", [128, KX, NCH], MM_DT, kind="ExternalInput")
        for n in range(N_CHUNKS)
    ]
    hd_ = [
        [
            nc.dram_tensor(f"h{d}_{n}", [128, KHF, NCH], MM_DT, kind="ExternalInput")
            for n in range(N_CHUNKS)
        ]
        for d in (0, 1)
    ]
    h8d_ = [
        [
            nc.dram_tensor(f"h8_{d}_{n}", [128, N_DR8, NCH], F8, kind="ExternalInput")
            for n in range(N_CHUNKS)
        ]
        for d in (0, 1)
    ]
    cd_ = [
        nc.dram_tensor(f"c{d}", [OUT_C, NS], F32, kind="ExternalInput")
        for d in (0, 1)
    ]
    # weights: [m_tile, partition(k%128), gate, k_tile, m_in_tile] — one
    # contiguous DMA per m-tile covering all 4 gates.
    wxd = nc.dram_tensor("wx", [M_TILES, 128, 4, KX, 128], MM_DT, kind="ExternalInput")
    whd = nc.dram_tensor("wh", [M_TILES, 128, 4, KHF, 128], MM_DT, kind="ExternalInput")
    wh8d = nc.dram_tensor(
        "wh8", [M_TILES, 128, 4, N_DR8, 128], F8, kind="ExternalInput"
    )
    biasd = nc.dram_tensor("bias", [128, 4 * M_TILES], F32, kind="ExternalInput")
    ctd = nc.dram_tensor("ct", [OUT_C, NS], F32, kind="ExternalOutput")
    htd = nc.dram_tensor("ht", [OUT_C, NS], F32, kind="ExternalOutput")

    with tile.TileContext(nc) as tc:
        with (
            tc.tile_pool(name="resident", bufs=1) as res_pool,
            tc.tile_pool(name="wx", bufs=3) as wx_pool,
            tc.tile_pool(name="wh", bufs=4) as wh_pool,
            tc.tile_pool(name="wh8", bufs=4) as wh8_pool,
            tc.tile_pool(name="psum", bufs=8, space="PSUM") as ps_pool,
            tc.tile_pool(name="xproj", bufs=20) as xp_pool,
            tc.tile_pool(name="gates", bufs=6) as g_pool,
            tc.tile_pool(name="cprev", bufs=3) as cp_pool,
            tc.tile_pool(name="tmp", bufs=2) as t_pool,
            tc.tile_pool(name="dirres", bufs=4) as dr_pool,
            tc.tile_pool(name="out", bufs=2) as o_pool,
        ):
            wx_tiles: dict = {}
            wh_tiles: dict = {}
            wh8_tiles: dict = {}

            def load_wx(mt):
                wx_tiles[mt] = wx_pool.tile(
                    [128, 4, KX, 128], MM_DT, tag="wx", name=f"wx_{mt}"
                )
                nc.sync.dma_start(wx_tiles[mt][:], wxd[mt])

            def load_wh8(mt):
                wh8_tiles[mt] = wh8_pool.tile(
                    [128, 4, N_DR8, 128], F8, tag="wh8", name=f"wh8_{mt}"
                )
                nc.sync.dma_start(wh8_tiles[mt][:], wh8d[mt])

            def load_wh(mt):
                wh_tiles[mt] = wh_pool.tile(
                    [128, 4, KHF, 128], MM_DT, tag="wh", name=f"wh_{mt}"
                )
                nc.sync.dma_start(wh_tiles[mt][:], whd[mt])

            def load_w(mt):
                load_wx(mt)
                load_wh8(mt)
                load_wh(mt)

            x_sb = [
                res_pool.tile([128, KX, NCH], MM_DT, tag=f"x{n}", name=f"x_sb{n}")
                for n in range(N_CHUNKS)
            ]
            h_sb = [
                [
                    res_pool.tile(
                        [128, KHF, NCH], MM_DT, tag=f"h{d}_{n}", name=f"h_sb{d}_{n}"
                    )
                    for n in range(N_CHUNKS)
                ]
                for d in (0, 1)
            ]
            h8_sb = [
                [
                    res_pool.tile(
                        [128, N_DR8, NCH], F8, tag=f"h8_{d}_{n}", name=f"h8_sb{d}_{n}"
                    )
                    for n in range(N_CHUNKS)
                ]
                for d in (0, 1)
            ]
            bias_sb = res_pool.tile([128, 4 * M_TILES], F32, tag="bias")

            # Startup: DMAs strictly in first-use order, one issue per
            # tensor-chunk. The PE's early work queue is px(0) then px(1)
            # (x-projections, needing only wx+x); the first hidden group
            # additionally needs wh8[0]+h8[d0]n0 (DR matmul comes first in
            # the group) then wh[0]+h[d0]n0. Meanwhile ~5us of throwaway
            # fp32 matmuls on the bias tile warm the PE HAM clock gate (idle
            # default is 1.2GHz; it takes ~3.4us of busy PE to unthrottle to
            # 2.4GHz) so the real stream starts warm.
            nc.sync.dma_start(bias_sb[:], biasd[:])
            load_wx(0)
            nc.sync.dma_start(x_sb[0][:], xd_[0][:])

            warm_ps = ps_pool.tile([128, NCH], F32, tag="ps", name="warm_ps")
            N_WARM = 48
            for i in range(N_WARM):
                nc.tensor.matmul(
                    warm_ps[:32, :32],
                    bias_sb[:, :32],
                    bias_sb[:, :32],
                    start=(i == 0),
                    stop=(i == N_WARM - 1),
                )

            load_wx(1)
            nc.sync.dma_start(x_sb[1][:], xd_[1][:])
            load_wh8(0)
            nc.sync.dma_start(h8_sb[0][0][:], h8d_[0][0][:])
            nc.sync.dma_start(h8_sb[1][0][:], h8d_[1][0][:])
            load_wh(0)
            nc.sync.dma_start(h_sb[0][0][:], hd_[0][0][:])
            nc.sync.dma_start(h_sb[1][0][:], hd_[1][0][:])
            nc.sync.dma_start(h8_sb[0][1][:], h8d_[0][1][:])
            nc.sync.dma_start(h_sb[0][1][:], hd_[0][1][:])
            nc.sync.dma_start(h8_sb[1][1][:], h8d_[1][1][:])
            nc.sync.dma_start(h_sb[1][1][:], hd_[1][1][:])
            load_wh8(1)
            load_wh(1)

            def px_phase(mt, ci, wxm):
                xp = []
                for g in range(4):
                    px = ps_pool.tile(
                        [128, NCH], F32, tag="ps", name=f"px_{mt}_{ci}_{g}"
                    )
                    for kt in range(KX):
                        nc.tensor.matmul(
                            px[:],
                            wxm[:, g, kt, :],
                            x_sb[ci][:, kt, :],
                            start=(kt == 0),
                            stop=(kt == KX - 1),
                        )
                    xpt = xp_pool.tile(
                        [128, NCH], F32, tag="xp", name=f"xp_{mt}_{ci}_{g}"
                    )
                    nc.scalar.activation(xpt[:], px[:], COPY)
                    xp.append(xpt)
                return xp

            def dir_phase(mt, ci, d, xp, whm, wh8m, msl):
                nsl = slice(ci * NCH, (ci + 1) * NCH)
                gt = []
                for g in range(4):
                    ps = ps_pool.tile(
                        [128, NCH], F32, tag="ps", name=f"ps_{mt}_{ci}_{d}_{g}"
                    )
                    # inject the shared x-projection, then accumulate the
                    # hidden projection on top of it: first k-tiles 0..1 as
                    # one fp8 DoubleRow pass, then k-tiles 2..7 in fp16
                    nc.vector.tensor_copy(ps[:], xp[g][:])
                    if N_DR8:
                        nc.tensor.matmul(
                            ps[:],
                            wh8m[:, g],
                            h8_sb[d][ci][:],
                            start=False,
                            stop=False,
                            skip_group_check=True,
                            perf_mode=mybir.MatmulPerfMode.DoubleRow,
                        )
                    for kh in range(KHF):
                        nc.tensor.matmul(
                            ps[:],
                            whm[:, g, kh, :],
                            h_sb[d][ci][:, kh, :],
                            start=False,
                            stop=(kh == KHF - 1),
                            skip_group_check=True,
                        )
                    gact = g_pool.tile(
                        [128, NCH], F32, tag="gate", name=f"gate_{mt}_{ci}_{d}_{g}"
                    )
                    nc.scalar.activation(
                        gact[:],
                        ps[:],
                        TANH if g == 2 else SIG,
                        bias=bias_sb[:, g * M_TILES + mt : g * M_TILES + mt + 1],
                    )
                    gt.append(gact)

                cp = cp_pool.tile([128, NCH], F32, tag="cp")
                nc.sync.dma_start(cp[:], cd_[d][msl, nsl])
                ig = t_pool.tile([128, NCH], F32, tag="ig")
                nc.vector.tensor_mul(ig[:], gt[0][:], gt[2][:])
                fc = t_pool.tile([128, NCH], F32, tag="fc")
                nc.vector.tensor_mul(fc[:], gt[1][:], cp[:])
                cnew = dr_pool.tile([128, NCH], F32, tag="cnew")
                nc.vector.tensor_add(cnew[:], ig[:], fc[:])
                tch = t_pool.tile([128, NCH], F32, tag="tch")
                nc.scalar.activation(tch[:], cnew[:], TANH)
                hnew = dr_pool.tile([128, NCH], F32, tag="hnew")
                nc.vector.tensor_mul(hnew[:], gt[3][:], tch[:])
                return cnew, hnew

            def combine(ci, msl, cdir, hdir):
                nsl = slice(ci * NCH, (ci + 1) * NCH)
                c0s = t_pool.tile([128, NCH], F32, tag="c0s")
                nc.vector.tensor_scalar_mul(c0s[:], cdir[0][:], ws0)
                ctt = o_pool.tile([128, NCH], F32, tag="ctt")
                nc.vector.scalar_tensor_tensor(
                    ctt[:], cdir[1][:], ws1, c0s[:], MULT, ADD
                )
                nc.sync.dma_start(ctd[msl, nsl], ctt[:])
                h0s = t_pool.tile([128, NCH], F32, tag="h0s")
                nc.vector.tensor_scalar_mul(h0s[:], hdir[0][:], ws0)
                htt = o_pool.tile([128, NCH], F32, tag="htt")
                nc.vector.scalar_tensor_tensor(
                    htt[:], hdir[1][:], ws1, h0s[:], MULT, ADD
                )
                nc.sync.dma_start(htd[msl, nsl], htt[:])

            # Software-pipelined x-projections: px(mt+1) is issued before
            # dirs(mt) so the PE has ~7us of weight/x-only work to chew on
            # whenever the hidden-projection inputs (wh, h, at startup) or
            # PSUM banks lag. At kernel start px(0)+px(1) = 64 MMs cover the
            # wh0/h0 DMA window that previously left a ~9us PE gap. These
            # early start=True groups also cover all 8 PSUM banks before any
            # start=False inject group runs (defined has_written state).
            xp_store: dict = {}

            def issue_px(mt):
                wxm = wx_tiles.pop(mt)
                for ci in range(N_CHUNKS):
                    xp_store[(mt, ci)] = px_phase(mt, ci, wxm)

            issue_px(0)
            for mt in range(M_TILES):
                msl = slice(mt * 128, (mt + 1) * 128)
                if mt + 2 < M_TILES:
                    load_w(mt + 2)
                whm = wh_tiles.pop(mt)
                wh8m = wh8_tiles.pop(mt)
                if mt + 1 < M_TILES:
                    issue_px(mt + 1)

                for ci in range(N_CHUNKS):
                    xps = xp_store.pop((mt, ci))
                    c0, h0 = dir_phase(mt, ci, 0, xps, whm, wh8m, msl)
                    c1, h1 = dir_phase(mt, ci, 1, xps, whm, wh8m, msl)
                    combine(ci, msl, [c0, c1], [h0, h1])

    nc.finalize()
    n_mm = sum(
        1 for i in nc.inst_map.values() if type(i).__name__ == "InstMatmult"
    )
    expected_mm = 2 * M_TILES * 4 * (KX + 2 * (KHF + (1 if N_DR8 else 0))) + 48
    assert n_mm == expected_mm, f"matmul count {n_mm} != {expected_mm}"
    return nc


_CACHE: dict = {}


def _get_nc(ws0: float, ws1: float):
    key = (ws0, ws1)
    if key not in _CACHE:
        _CACHE.clear()
        _CACHE[key] = _build(ws0, ws1)
    return _CACHE[key]


F8_NP = _mld.float8_e4m3  # TRN fp8e4: IEEE e4m3, max normal +-240


def _prep_w(w: np.ndarray, kt: int, np_dt=None) -> np.ndarray:
    """(K, OUT_C)-transposed weight rows -> [m_tile, partition, k_tile, m_in_tile]."""
    wT = w
    k = wT.shape[0]
    assert k == kt * 128
    r = wT.reshape(kt, 128, M_TILES, 128)  # [ktile, p, mtile, mi]
    return np.ascontiguousarray(
        r.transpose(2, 1, 0, 3).astype(np_dt if np_dt is not None else MM_NP)
    )


def _prep_rhs(a: np.ndarray, kt: int, np_dt=None) -> np.ndarray:
    """(K, n) activation rows -> [partition, k_tile, n]."""
    k, n = a.shape
    assert k == kt * 128
    return np.ascontiguousarray(
        a.reshape(kt, 128, n).transpose(1, 0, 2).astype(
            np_dt if np_dt is not None else MM_NP
        )
    )


def run(inputs: dict, trace: bool = False, trace_kwargs: dict | None = None):
    x = np.asarray(inputs["x"], dtype=np.float32)
    ws = np.asarray(inputs["weighted_sum"], dtype=np.float32)
    ws0, ws1 = float(ws[0]), float(ws[1])
    nc = _get_nc(ws0, ws1)

    # weight hosts: [m_tile, partition, gate, k_tile, m_in_tile]
    wx_host = np.ascontiguousarray(np.stack(
        [_prep_w(np.ascontiguousarray(np.asarray(inputs[k], dtype=np.float32).T), KX)
         for k in ("w_ii", "w_if", "w_ig", "w_io")],
        axis=2,
    ))
    whT = [
        np.ascontiguousarray(np.asarray(inputs[k], dtype=np.float32).T)
        for k in ("w_hi", "w_hf", "w_hg", "w_ho")
    ]
    kc = N_DR8 * 128
    wh_host = np.ascontiguousarray(np.stack([_prep_w(w[kc:], KHF) for w in whT], axis=2))
    wh8_host = np.ascontiguousarray(
        np.stack([_prep_w(w[:kc], N_DR8, F8_NP) for w in whT], axis=2)
    )
    bias_host = np.concatenate(
        [np.asarray(inputs[k], dtype=np.float32).reshape(M_TILES, 128).T
         for k in ("b_i", "b_f", "b_g", "b_o")],
        axis=1,
    )
    bias_host = np.ascontiguousarray(bias_host)

    h0 = np.asarray(inputs["h_prev_dim0"], dtype=np.float32)
    h1 = np.asarray(inputs["h_prev_dim1"], dtype=np.float32)
    c0 = np.asarray(inputs["c_prev_dim0"], dtype=np.float32)
    c1 = np.asarray(inputs["c_prev_dim1"], dtype=np.float32)

    in_maps = []
    for core in range(N_CORES):
        csl = slice(core * NS, (core + 1) * NS)
        m = {
            "c0": np.ascontiguousarray(c0[:, csl]),
            "c1": np.ascontiguousarray(c1[:, csl]),
            "wx": wx_host,
            "wh": wh_host,
            "wh8": wh8_host,
            "bias": bias_host,
        }
        xr = _prep_rhs(x[:, csl], KX)
        hr = [_prep_rhs(h0[kc:, csl], KHF), _prep_rhs(h1[kc:, csl], KHF)]
        h8r = [
            _prep_rhs(h0[:kc, csl], N_DR8, F8_NP),
            _prep_rhs(h1[:kc, csl], N_DR8, F8_NP),
        ]
        for n in range(N_CHUNKS):
            nsl = slice(n * NCH, (n + 1) * NCH)
            m[f"x{n}"] = np.ascontiguousarray(xr[:, :, nsl])
            for d in (0, 1):
                m[f"h{d}_{n}"] = np.ascontiguousarray(hr[d][:, :, nsl])
                m[f"h8_{d}_{n}"] = np.ascontiguousarray(h8r[d][:, :, nsl])
        in_maps.append(m)

    res = run_bass_kernel_spmd(
        nc,
        in_maps,
        list(range(N_CORES)),
        trace=trace,
        **(trace_kwargs or {}),
    )
    ct = np.concatenate([res.results[c]["ct"] for c in range(N_CORES)], axis=1)
    ht = np.concatenate([res.results[c]["ht"] for c in range(N_CORES)], axis=1)
    return (ct, ht), res


def kernel(**inputs) -> tuple:
    (ct, ht), _ = run(inputs)
    return ct, ht

